# revision 10
# baseline (speedup 1.0000x reference)
"""AvgPoolingMerger Trainium2 kernel.

Per sample: gather G*K rows from a length-L window of hidden_states,
mean-reduce the valid ones per group, right-align kept groups into a
[MAX_TOKENS, D] output.

Formulation: out[b] = C'[b] @ U[b] where U[b] is the sample's deduplicated
set of gathered rows (~900 unique of 1536 window rows, fetched by indirect
DMA in 128-row chunks) and C'[b] is a host-built sparse [MAX_TOKENS, n_u]
count matrix over gather positions (integer counts, bf16-exact; rows placed
at their right-aligned destination).  The TensorEngine does the segment
reduce + scatter in one matmul chain per (t-chunk, d-chunk); the 1/cnt mean
scaling runs on the VectorEngine with per-partition f32 scalars while
copying PSUM->SBUF.  Rows are bf16 (host-cast) to halve HBM traffic; PSUM
accumulation is f32.  Pure data parallel across 8 cores, 4 samples each.
"""

import ml_dtypes
import numpy as np

import concourse.bass as bass
import concourse.mybir as mybir
from concourse.tile import TileContext
from concourse.bass_utils import run_bass_kernel_spmd

B, S, D = 32, 2048, 1536
G, K = 150, 9
L = 1536
MAX_TOKENS = 150
N_CORES = 8
BPC = B // N_CORES          # samples per core
DSPLIT = 512                # PSUM bank free-dim (fp32)
NDC = D // DSPLIT
T_CHUNKS = [(0, 128), (128, MAX_TOKENS - 128)]

F32 = mybir.dt.float32
BF16 = mybir.dt.bfloat16
I32 = mybir.dt.int32


def _split_excess_waits(nc):
    """Walrus (CoreV3) rejects instructions carrying more than one semaphore
    wait.  Hoist the excess onto same-engine NOPs inserted immediately before
    the offending instruction (engines execute their stream in order, so a
    preceding wait is equivalent)."""
    for f in nc.m.functions:
        for bb in f.blocks:
            insts = list(bb.instructions)
            out, changed = [], False
            for inst in insts:
                si = inst.sync_info
                if si is not None and si.on_wait and len(si.on_wait) > 1:
                    waits = list(si.on_wait)
                    extra, keep = waits[:-1], waits[-1:]
                    for w in extra:
                        nop = nc.engines[inst.engine].nop(nofuse=True).ins
                        # nop() appended itself to the current block; pull it
                        # back out of wherever it landed.
                        for f2 in nc.m.functions:
                            for bb2 in f2.blocks:
                                l2 = list(bb2.instructions)
                                if l2 and l2[-1].name == nop.name:
                                    bb2.instructions = l2[:-1]
                        nop.sync_info = mybir.SyncInfo(on_wait=[w], on_update=[])
                        out.append(nop)
                    inst.sync_info = mybir.SyncInfo(
                        on_wait=keep, on_update=list(si.on_update or [])
                    )
                    changed = True
                out.append(inst)
            if changed:
                bb.instructions = out


def _build(nc, nch):
    hsd = nc.declare_dram_parameter("hsd", [BPC * S, D], BF16, isOutput=False)
    gi = nc.declare_dram_parameter("gi", [BPC, 128, nch], I32, isOutput=False)
    ct = nc.declare_dram_parameter("ct", [BPC, 128, nch * MAX_TOKENS], BF16, isOutput=False)
    invc = nc.declare_dram_parameter("invc", [BPC, MAX_TOKENS], F32, isOutput=False)
    out = nc.declare_dram_parameter("out", [BPC, MAX_TOKENS, D], F32, isOutput=True)

    with (
        TileContext(nc) as tc,
        tc.tile_pool(name="w", bufs=2 * nch + 2) as wp,
        tc.tile_pool(name="c", bufs=2) as cp,
        tc.tile_pool(name="o", bufs=2) as op,
        tc.tile_pool(name="ic", bufs=2) as ip,
        tc.tile_pool(name="gx", bufs=2) as gp,
        tc.tile_pool(name="ps", bufs=8, space="PSUM") as pp,
    ):
        for b in range(BPC):
            idx_t = gp.tile([128, nch], I32, tag="gi")
            nc.sync.dma_start(out=idx_t[:], in_=gi[b])
            ct_t = cp.tile([128, nch * MAX_TOKENS], BF16, tag="ct")
            nc.sync.dma_start(out=ct_t[:], in_=ct[b])
            ic0 = ip.tile([128, 1], F32, tag="ic0")
            ic1 = ip.tile([32, 1], F32, tag="ic1")
            nc.sync.dma_start(out=ic0[:], in_=invc[b, 0:128, None])
            nc.sync.dma_start(
                out=ic1[: MAX_TOKENS - 128, :], in_=invc[b, 128:MAX_TOKENS, None]
            )
            ics = [ic0, ic1]

            psums = {}
            for ti, (t0, tsz) in enumerate(T_CHUNKS):
                for dc in range(NDC):
                    psums[(ti, dc)] = pp.tile(
                        [128, DSPLIT], F32, tag="ps", name=f"ps_{b}_{ti}_{dc}"
                    )

            w_ts = []
            for c in range(nch):
                w_t = wp.tile([128, D], BF16, tag="w", name=f"w_{b}_{c}")
                nc.gpsimd.indirect_dma_start(
                    out=w_t[:],
                    out_offset=None,
                    in_=hsd[:],
                    in_offset=bass.IndirectOffsetOnAxis(ap=idx_t[:, c : c + 1], axis=0),
                )
                w_ts.append(w_t)

            st0 = op.tile([128, D], F32, tag="st0")
            st1 = op.tile([32, D], F32, tag="st1")
            stages = [st0, st1]
            # (t, d)-chain outer, gather-chunk inner: each PSUM tile finishes
            # its accumulation early and drains on DVE while PE streams the
            # next chain, instead of all six draining at sample end.
            for ti, (t0, tsz) in enumerate(T_CHUNKS):
                for dc in range(NDC):
                    for c in range(nch):
                        nc.tensor.matmul(
                            out=psums[(ti, dc)][:tsz, :],
                            lhsT=ct_t[:, c * MAX_TOKENS + t0 : c * MAX_TOKENS + t0 + tsz],
                            rhs=w_ts[c][:, dc * DSPLIT : (dc + 1) * DSPLIT],
                            start=(c == 0),
                            stop=(c == nch - 1),
                        )
                    nc.vector.tensor_scalar_mul(
                        out=stages[ti][:tsz, dc * DSPLIT : (dc + 1) * DSPLIT],
                        in0=psums[(ti, dc)][:tsz, :],
                        scalar1=ics[ti][:tsz, :],
                    )
            nc.sync.dma_start(out=out[b, 0:128, :], in_=st0[:])
            nc.sync.dma_start(out=out[b, 128:MAX_TOKENS, :], in_=st1[: MAX_TOKENS - 128, :])


def _host_prep(pr, idx, rem):
    """Mirror of the reference's per-sample index math.  Returns the window
    start, the deduplicated window-row list u, the count matrix C' over
    gather positions, the per-row 1/cnt vector and the attention row."""
    start = int(pr[0])
    end = int(pr[1])
    start = max(0, min(start, S - L))
    mask = idx != -1
    rem_valid = rem != -1
    isin = ((idx[..., None] == rem[None, None, :]) & rem_valid[None, None, :]).any(-1)
    mask = mask & ~isin
    keep = mask.any(-1)
    cnt = mask.sum(-1)
    gidx = np.where(idx >= 0, start + idx, end + 1 + idx)
    wrel = np.clip(gidx - start, 0, L - 1)
    n_kept = int(keep.sum())
    pos = MAX_TOKENS - n_kept + np.cumsum(keep) - 1
    dest = np.where(keep, pos, MAX_TOKENS)
    u = np.unique(wrel)                      # sorted unique window rows
    j = np.searchsorted(u, wrel)             # gather position per (g, k)
    Cp = np.zeros((MAX_TOKENS + 1, len(u)), np.float32)
    np.add.at(Cp, (np.repeat(dest, K), j.ravel()), 1.0)
    inv = np.ones((MAX_TOKENS + 1,), np.float32)
    inv[dest] = (1.0 / np.maximum(cnt, 1)).astype(np.float32)
    attn = np.zeros((MAX_TOKENS,), np.int32)
    attn[dest[keep]] = 1
    return start, u, Cp[:MAX_TOKENS], inv[:MAX_TOKENS], attn


_CACHED_NC = {}


def _get_nc(nch):
    if nch not in _CACHED_NC:
        nc = bass.Bass()
        _build(nc, nch)
        _split_excess_waits(nc)
        _CACHED_NC[nch] = nc
    return _CACHED_NC[nch]


def _prep_in_maps(hidden_states, patch_range_list, patch_indices_list_list,
                  remove_index_list_list):
    hs = np.asarray(hidden_states, dtype=np.float32)
    pr = np.asarray(patch_range_list, dtype=np.int64)
    idx = np.asarray(patch_indices_list_list, dtype=np.int64)
    rem = np.asarray(remove_index_list_list, dtype=np.int64)

    preps = [_host_prep(pr[b], idx[b], rem[b]) for b in range(B)]
    nch = max(-(-len(p[1]) // 128) for p in preps)

    gis = np.zeros((B, 128, nch), np.int32)
    cts = np.zeros((B, 128, nch * MAX_TOKENS), ml_dtypes.bfloat16)
    invcs = np.empty((B, MAX_TOKENS), np.float32)
    attn = np.empty((B, MAX_TOKENS), np.int32)
    for b in range(B):
        s0, u, Cp, inv, a = preps[b]
        bl = b % BPC
        n_u = len(u)
        rows = np.full((nch * 128,), bl * S, np.int64)      # pads hit row 0
        rows[:n_u] = bl * S + s0 + u
        gis[b] = rows.reshape(nch, 128).T                   # gi[p, c] = rows[c*128+p]
        Cpad = np.zeros((MAX_TOKENS, nch * 128), np.float32)
        Cpad[:, :n_u] = Cp
        # ct[p, c*T + t] = Cpad[t, c*128 + p]; integer counts, bf16-exact
        cts[b] = Cpad.T.reshape(nch, 128, MAX_TOKENS).transpose(1, 0, 2).reshape(128, -1)
        invcs[b] = inv
        attn[b] = a

    hsb = hs.reshape(N_CORES, BPC * S, D)
    in_maps = [
        {
            "hsd": hsb[i].astype(ml_dtypes.bfloat16),
            "gi": np.ascontiguousarray(gis[i * BPC : (i + 1) * BPC]),
            "ct": np.ascontiguousarray(cts[i * BPC : (i + 1) * BPC]),
            "invc": np.ascontiguousarray(invcs[i * BPC : (i + 1) * BPC]),
        }
        for i in range(N_CORES)
    ]
    return in_maps, attn, nch


def kernel(
    hidden_states,
    attention_mask,
    image_grid_thw,
    patch_range_list,
    patch_indices_list_list,
    remove_index_list_list,
):
    am_dtype = np.asarray(attention_mask).dtype
    in_maps, attn, nch = _prep_in_maps(
        hidden_states, patch_range_list, patch_indices_list_list, remove_index_list_list
    )
    nc = _get_nc(nch)
    res = run_bass_kernel_spmd(nc, in_maps, core_ids=list(range(N_CORES)))
    outputs = np.concatenate([res.results[i]["out"] for i in range(N_CORES)], axis=0)
    return outputs, attn.astype(am_dtype)


# revision 11
# speedup vs baseline: 1.0664x; 1.0664x over previous
"""AvgPoolingMerger Trainium2 kernel.

Per sample: gather G*K rows from a length-L window of hidden_states,
mean-reduce the valid ones per group, right-align kept groups into a
[MAX_TOKENS, D] output.

Formulation: out[b] = C'[b] @ U[b] where U[b] is the sample's deduplicated
set of gathered rows (~900 unique of 1536 window rows, fetched by indirect
DMA in 128-row chunks) and C'[b] is a host-built sparse [MAX_TOKENS, n_u]
count matrix over gather positions (integer counts, bf16-exact; rows placed
at their right-aligned destination).  The TensorEngine does the segment
reduce + scatter in one matmul chain per (t-chunk, d-chunk); the 1/cnt mean
scaling runs on the VectorEngine with per-partition f32 scalars while
copying PSUM->SBUF.  Rows are bf16 (host-cast) to halve HBM traffic; PSUM
accumulation is f32.  Pure data parallel across 8 cores, 4 samples each.
"""

import ml_dtypes
import numpy as np

import concourse.bass as bass
import concourse.mybir as mybir
from concourse.tile import TileContext
from concourse.bass_utils import run_bass_kernel_spmd

B, S, D = 32, 2048, 1536
G, K = 150, 9
L = 1536
MAX_TOKENS = 150
N_CORES = 8
BPC = B // N_CORES          # samples per core
DSPLIT = 512                # PSUM bank free-dim (fp32)
NDC = D // DSPLIT
T_CHUNKS = [(0, 128), (128, MAX_TOKENS - 128)]

F32 = mybir.dt.float32
BF16 = mybir.dt.bfloat16
I32 = mybir.dt.int32


def _split_excess_waits(nc):
    """Walrus (CoreV3) rejects instructions carrying more than one semaphore
    wait.  Hoist the excess onto same-engine NOPs inserted immediately before
    the offending instruction (engines execute their stream in order, so a
    preceding wait is equivalent)."""
    for f in nc.m.functions:
        for bb in f.blocks:
            insts = list(bb.instructions)
            out, changed = [], False
            for inst in insts:
                si = inst.sync_info
                if si is not None and si.on_wait and len(si.on_wait) > 1:
                    waits = list(si.on_wait)
                    extra, keep = waits[:-1], waits[-1:]
                    for w in extra:
                        nop = nc.engines[inst.engine].nop(nofuse=True).ins
                        # nop() appended itself to the current block; pull it
                        # back out of wherever it landed.
                        for f2 in nc.m.functions:
                            for bb2 in f2.blocks:
                                l2 = list(bb2.instructions)
                                if l2 and l2[-1].name == nop.name:
                                    bb2.instructions = l2[:-1]
                        nop.sync_info = mybir.SyncInfo(on_wait=[w], on_update=[])
                        out.append(nop)
                    inst.sync_info = mybir.SyncInfo(
                        on_wait=keep, on_update=list(si.on_update or [])
                    )
                    changed = True
                out.append(inst)
            if changed:
                bb.instructions = out


def _build(nc, nch):
    hsd = nc.declare_dram_parameter("hsd", [BPC * S, D], BF16, isOutput=False)
    gi = nc.declare_dram_parameter("gi", [BPC, 128, nch], I32, isOutput=False)
    ct = nc.declare_dram_parameter("ct", [BPC, 128, nch * MAX_TOKENS], BF16, isOutput=False)
    invc = nc.declare_dram_parameter("invc", [BPC, MAX_TOKENS], F32, isOutput=False)
    out = nc.declare_dram_parameter("out", [BPC, MAX_TOKENS, D], F32, isOutput=True)

    with (
        TileContext(nc) as tc,
        tc.tile_pool(name="w", bufs=2 * nch + 2) as wp,
        tc.tile_pool(name="c", bufs=2) as cp,
        tc.tile_pool(name="o", bufs=2) as op,
        tc.tile_pool(name="ic", bufs=2) as ip,
        tc.tile_pool(name="gx", bufs=2) as gp,
        tc.tile_pool(name="ps", bufs=8, space="PSUM") as pp,
    ):
        for b in range(BPC):
            idx_t = gp.tile([128, nch], I32, tag="gi")
            nc.sync.dma_start(out=idx_t[:], in_=gi[b])
            ct_t = cp.tile([128, nch * MAX_TOKENS], BF16, tag="ct")
            nc.sync.dma_start(out=ct_t[:], in_=ct[b])
            ic0 = ip.tile([128, 1], F32, tag="ic0")
            ic1 = ip.tile([32, 1], F32, tag="ic1")
            nc.sync.dma_start(out=ic0[:], in_=invc[b, 0:128, None])
            nc.sync.dma_start(
                out=ic1[: MAX_TOKENS - 128, :], in_=invc[b, 128:MAX_TOKENS, None]
            )
            ics = [ic0, ic1]

            psums = {}
            for ti, (t0, tsz) in enumerate(T_CHUNKS):
                for dc in range(NDC):
                    psums[(ti, dc)] = pp.tile(
                        [128, DSPLIT], F32, tag="ps", name=f"ps_{b}_{ti}_{dc}"
                    )

            w_ts = []
            for c in range(nch):
                w_t = wp.tile([128, D], BF16, tag="w", name=f"w_{b}_{c}")
                nc.gpsimd.indirect_dma_start(
                    out=w_t[:],
                    out_offset=None,
                    in_=hsd[:],
                    in_offset=bass.IndirectOffsetOnAxis(ap=idx_t[:, c : c + 1], axis=0),
                )
                w_ts.append(w_t)

            for c in range(nch):
                for ti, (t0, tsz) in enumerate(T_CHUNKS):
                    for dc in range(NDC):
                        nc.tensor.matmul(
                            out=psums[(ti, dc)][:tsz, :],
                            lhsT=ct_t[:, c * MAX_TOKENS + t0 : c * MAX_TOKENS + t0 + tsz],
                            rhs=w_ts[c][:, dc * DSPLIT : (dc + 1) * DSPLIT],
                            start=(c == 0),
                            stop=(c == nch - 1),
                        )

            st0 = op.tile([128, D], F32, tag="st0")
            st1 = op.tile([32, D], F32, tag="st1")
            stages = [st0, st1]
            for ti, (t0, tsz) in enumerate(T_CHUNKS):
                for dc in range(NDC):
                    nc.vector.tensor_scalar_mul(
                        out=stages[ti][:tsz, dc * DSPLIT : (dc + 1) * DSPLIT],
                        in0=psums[(ti, dc)][:tsz, :],
                        scalar1=ics[ti][:tsz, :],
                    )
            nc.sync.dma_start(out=out[b, 0:128, :], in_=st0[:])
            nc.sync.dma_start(out=out[b, 128:MAX_TOKENS, :], in_=st1[: MAX_TOKENS - 128, :])


def _host_prep(pr, idx, rem):
    """Mirror of the reference's per-sample index math.  Returns the window
    start, the deduplicated window-row list u, the count matrix C' over
    gather positions, the per-row 1/cnt vector and the attention row."""
    start = int(pr[0])
    end = int(pr[1])
    start = max(0, min(start, S - L))
    mask = idx != -1
    rem_valid = rem != -1
    isin = ((idx[..., None] == rem[None, None, :]) & rem_valid[None, None, :]).any(-1)
    mask = mask & ~isin
    keep = mask.any(-1)
    cnt = mask.sum(-1)
    gidx = np.where(idx >= 0, start + idx, end + 1 + idx)
    wrel = np.clip(gidx - start, 0, L - 1)
    n_kept = int(keep.sum())
    pos = MAX_TOKENS - n_kept + np.cumsum(keep) - 1
    dest = np.where(keep, pos, MAX_TOKENS)
    u = np.unique(wrel)                      # sorted unique window rows
    j = np.searchsorted(u, wrel)             # gather position per (g, k)
    Cp = np.zeros((MAX_TOKENS + 1, len(u)), np.float32)
    np.add.at(Cp, (np.repeat(dest, K), j.ravel()), 1.0)
    inv = np.ones((MAX_TOKENS + 1,), np.float32)
    inv[dest] = (1.0 / np.maximum(cnt, 1)).astype(np.float32)
    attn = np.zeros((MAX_TOKENS,), np.int32)
    attn[dest[keep]] = 1
    return start, u, Cp[:MAX_TOKENS], inv[:MAX_TOKENS], attn


_CACHED_NC = {}


def _get_nc(nch):
    if nch not in _CACHED_NC:
        nc = bass.Bass()
        _build(nc, nch)
        _split_excess_waits(nc)
        _CACHED_NC[nch] = nc
    return _CACHED_NC[nch]


def _prep_in_maps(hidden_states, patch_range_list, patch_indices_list_list,
                  remove_index_list_list):
    hs = np.asarray(hidden_states, dtype=np.float32)
    pr = np.asarray(patch_range_list, dtype=np.int64)
    idx = np.asarray(patch_indices_list_list, dtype=np.int64)
    rem = np.asarray(remove_index_list_list, dtype=np.int64)

    preps = [_host_prep(pr[b], idx[b], rem[b]) for b in range(B)]
    nch = max(-(-len(p[1]) // 128) for p in preps)

    gis = np.zeros((B, 128, nch), np.int32)
    cts = np.zeros((B, 128, nch * MAX_TOKENS), ml_dtypes.bfloat16)
    invcs = np.empty((B, MAX_TOKENS), np.float32)
    attn = np.empty((B, MAX_TOKENS), np.int32)
    for b in range(B):
        s0, u, Cp, inv, a = preps[b]
        bl = b % BPC
        n_u = len(u)
        rows = np.full((nch * 128,), bl * S, np.int64)      # pads hit row 0
        rows[:n_u] = bl * S + s0 + u
        gis[b] = rows.reshape(nch, 128).T                   # gi[p, c] = rows[c*128+p]
        Cpad = np.zeros((MAX_TOKENS, nch * 128), np.float32)
        Cpad[:, :n_u] = Cp
        # ct[p, c*T + t] = Cpad[t, c*128 + p]; integer counts, bf16-exact
        cts[b] = Cpad.T.reshape(nch, 128, MAX_TOKENS).transpose(1, 0, 2).reshape(128, -1)
        invcs[b] = inv
        attn[b] = a

    hsb = hs.reshape(N_CORES, BPC * S, D)
    in_maps = [
        {
            "hsd": hsb[i].astype(ml_dtypes.bfloat16),
            "gi": np.ascontiguousarray(gis[i * BPC : (i + 1) * BPC]),
            "ct": np.ascontiguousarray(cts[i * BPC : (i + 1) * BPC]),
            "invc": np.ascontiguousarray(invcs[i * BPC : (i + 1) * BPC]),
        }
        for i in range(N_CORES)
    ]
    return in_maps, attn, nch


def kernel(
    hidden_states,
    attention_mask,
    image_grid_thw,
    patch_range_list,
    patch_indices_list_list,
    remove_index_list_list,
):
    am_dtype = np.asarray(attention_mask).dtype
    in_maps, attn, nch = _prep_in_maps(
        hidden_states, patch_range_list, patch_indices_list_list, remove_index_list_list
    )
    nc = _get_nc(nch)
    res = run_bass_kernel_spmd(nc, in_maps, core_ids=list(range(N_CORES)))
    outputs = np.concatenate([res.results[i]["out"] for i in range(N_CORES)], axis=0)
    return outputs, attn.astype(am_dtype)
